# revision 37
# baseline (speedup 1.0000x reference)
"""Trainium2 Bass kernel for the sparse-attention CompiledTransformerLayer.

Math (derived from the reference):
  c0 = rowsum(mask0); attended = (mask0 @ x[:,:,0:16]) * r/(1-r), r = 1/(1+c0)
  out ch16:32 = attended @ W_o0.T
  out ch32    = c1 * W_o1[0,0], c1 = rowsum(mask1)
  out ch48:64 = a + b; 64:80 = a*b; 80:96 = (a > b), a = x ch0:16, b = ch16:32
  all other channels pass through from x (merged on the host).

Sharding: 8 cores = 4 batches x 2 query-halves (1024 queries each).

Key tricks:
  - nibble packing: the host packs BOTH masks for two adjacent keys into one
    byte  p = m0[2j] + 2*m0[2j+1] + 4*(m1[2j]+m1[2j+1]).  fp8e4 decodes bytes
    0..15 exactly as k*2^-9, so one transposed load (1MB/core instead of 4MB)
    carries all mask information.
  - the packed bytes are DMA-transposed as uint16 (HWDGE xbar), then two DVE
    shift/and ops recover the mask0 even/odd key bit-planes (values 0x00/0x01
    = fp8 0 / 2^-9).
  - matmuls are flipped vs the usual attention layout: a 128x128 mask^T block
    is the *stationary* operand and the 34-wide value vector
    [u_hi | u_lo | ones | gones] is the *moving* operand, so each matmul only
    streams 34 columns.  PSUM ends up as [query, channel] - no transposes.
  - u = x[:,:,0:16] @ W_o0.T is precomputed on the host, split hi+lo in bf16
    (scaled by 512 to cancel the fp8 2^-9), giving ~f32 matmul precision.
  - c1 falls out linearly: a raw-packed-byte matmul against ones gives
    T = g + 4*c1 where g = c0_even + 2*c0_odd comes from the 34th moving
    column (weight 1 on the even plane, 2 on the odd plane).
"""
import sys
sys.path.insert(0, "/opt/trn_rl_repo")
import numpy as np
import ml_dtypes

import concourse.bass as bass
import concourse.mybir as mybir
from concourse import tile
from concourse.bass_utils import run_bass_kernel_spmd
from concourse.vector_clock import ScopedClock, VectorClock
from concourse.tile import add_dep_helper

B, S, D = 4, 2048, 128
QH = S // 2              # queries per core
NCH = 4                  # transpose chunks (128 u16 cols = 512 keys each)
NIB = QH // 128          # query blocks per core (8)
DT = mybir.dt
AL = mybir.AluOpType

# walrus codegen rejects instructions with many sem waits; the Tile tail
# drain accumulates one wait per touched proc. Emit one single-wait drain
# per proc instead.
def _patched_dab(self, tick_clock, wait_clock):
    ticks = list(tick_clock.global_clock)
    for i, t in enumerate(ticks):
        if t <= 0:
            continue
        part = [t if j == i else 0 for j, t in enumerate(ticks)]
        d = self.nc.sync.drain()
        wait_clock.add_sem_waits(d.ins, ScopedClock({None: VectorClock(part)}))
    self.nc.sync.drain()
    self.nc.all_engine_barrier()
    popped = self.nc._tile_sem_poison_stack.pop()
    assert popped is self._sem_poison
    self.nc.clear_and_free_semaphores(list(self.sems.allocated().values()))
    self.nc.all_engine_barrier()
tile.TileContext._drain_and_barrier = _patched_dab


def _build_program():
    nc = bass.Bass()
    mp_d = nc.declare_dram_parameter("mp", [QH, S // 4], DT.uint16, isOutput=False)
    blob_d = nc.declare_dram_parameter("blob", [848, 128], DT.uint16, isOutput=False)
    outa_d = nc.declare_dram_parameter("outa", [128, NIB, 48], DT.float32, isOutput=True)
    outb_d = nc.declare_dram_parameter("outb", [128, NIB, 17], DT.float32, isOutput=True)

    # (chunk, row0, rows, ib0): last key-chunk split into query halves so the
    # final extract+matmul straggler after the last transpose is half-sized
    PIECES = [(0, 0, QH, 0), (1, 0, QH, 0), (2, 0, QH, 0),
              (3, 0, QH // 2, 0), (3, QH // 2, QH // 2, NIB // 2)]

    with tile.TileContext(nc) as tc, \
         tc.tile_pool(name="const", bufs=1) as cpool, \
         tc.tile_pool(name="masks", bufs=1) as mpool, \
         tc.tile_pool(name="work", bufs=1) as wpool, \
         tc.tile_pool(name="ps", bufs=1, space="PSUM") as ps:

        # every input load goes through the xbar (transpose DMA): mixing
        # regular and transpose DMAs forces a serializing mode-switch fence
        # between each pair, so the small inputs ride one pre-transposed blob.
        blob_t = mpool.tile([128, 848], DT.uint16, tag="blob", name="blob")
        nc.sync.dma_start(blob_t[:], blob_d[:], transpose=True)
        # DVE-launder the blob so every consumer dep collapses onto DVE sems
        blob2 = cpool.tile([128, 836], DT.uint16)
        nc.vector.tensor_copy(blob2[:], blob_t[:, 0:836])
        w3 = blob2[:, 0:576].bitcast(DT.bfloat16).rearrange(
            "p (t a b c d) -> p t a b c d", t=NCH, a=2, b=2, c=2, d=18)
        xq = blob2[:, 576:832].bitcast(DT.float32).rearrange(
            "p (i c) -> p i c", i=NIB, c=16)
        cwo1 = blob2[:, 832:834].bitcast(DT.float32)
        cone = blob2[:, 834:835].bitcast(DT.bfloat16)

        # start=True resets psum at larger-than-region granularity, so zero the
        # whole accumulator with one dummy all-zeros matmul and use start=False
        # (pure accumulate) for every real matmul.  One tile holds S (cols
        # 0:16 att, 16 c0) and C (col 17, T-g) so all writers are PE.
        P_ps = ps.tile([128, NIB, 18], DT.float32, tag="P", name="P")
        zmv = cpool.tile([128, 160], DT.bfloat16)
        nc.vector.memset(zmv[:], 0.0)
        nc.tensor.matmul(P_ps[:].rearrange("p a b -> p (a b)"), zmv[:, 0:128],
                         zmv[:, 0:NIB * 18], start=True, stop=False,
                         skip_group_check=True)

        last_tdma = None
        for pi, (t, row0, rows, ib0) in enumerate(PIECES):
            nib = rows // 128
            mt = mpool.tile([128, rows], DT.uint16, tag=f"mt{pi}", name=f"mt{pi}")
            last_tdma = nc.sync.dma_start(
                mt[:], mp_d[row0:row0 + rows, 128 * t:128 * (t + 1)],
                transpose=True)
            # bit-plane extracts: even keys = bit0, odd keys = bit1 (per byte)
            ev = mpool.tile([128, rows], DT.uint16, tag=f"ev{pi}", name=f"ev{pi}")
            nc.vector.tensor_scalar(ev[:], mt[:], 0x0101, 0, AL.bitwise_and,
                                    AL.bitwise_or)
            od = mpool.tile([128, rows], DT.uint16, tag=f"od{pi}", name=f"od{pi}")
            odx = nc.vector.tensor_scalar(od[:], mt[:], 1, 0x0101,
                                          AL.logical_shift_right, AL.bitwise_and)

            mr = mt[:].bitcast(DT.float8e4).rearrange("p (i two) -> p i two", two=2)
            evr = ev[:].bitcast(DT.float8e4).rearrange("p (i two) -> p i two", two=2)
            odr = od[:].bitcast(DT.float8e4).rearrange("p (i two) -> p i two", two=2)

            last = (pi == len(PIECES) - 1)
            for par in range(2):
                for e, pl in ((0, evr), (1, odr)):
                    stops = (last and par == 1 and e == 1)
                    # 18-wide moving: [u_hi|u_lo (16) | ones->c0 | -g ones]
                    for hl in range(2):
                        mv = w3[:, t, par, e, hl, :]
                        for k in range(nib):
                            nc.tensor.matmul(
                                P_ps[:, ib0 + k, 0:18],
                                pl[:, 128 * k:128 * (k + 1), par], mv,
                                start=False,
                                stop=(stops and hl == 1 and k == nib - 1),
                                skip_group_check=True)
                # raw packed bytes vs ones: C += g + 4*c1
                for k in range(nib):
                    cmm = nc.tensor.matmul(
                        P_ps[:, ib0 + k, 17:18], mr[:, 128 * k:128 * (k + 1), par],
                        cone[:, 0:1],
                        start=False,
                        stop=(last and par == 1 and k == nib - 1),
                        skip_group_check=True)
                    # route deps through the chunk's DVE extract so the wait
                    # set collapses to a single DVE sem (covers mt + cone)
                    add_dep_helper(cmm.ins, odx.ins, reason="chunk ready")

        # ---- tail: scale + MLP, all in [128 queries, NIB, ch] layout ------
        # attended = S * w with w = 1/max(c0, 1): exact for c0 >= 1, and for
        # c0 == 0 S is exactly 0 so any finite w gives the reference 0.
        mcol = wpool.tile([128, NIB], DT.float32, tag="mcol")
        nc.vector.tensor_scalar_max(mcol[:], P_ps[:, :, 16], 1.0)
        wcol = wpool.tile([128, NIB], DT.float32, tag="wcol")
        nc.vector.reciprocal(wcol[:], mcol[:])

        # otA: [atts 16 | a+b 16 | a*b 16]; otB: [count 1 | a>b 16]
        otA = wpool.tile([128, NIB, 48], DT.float32, tag="otA")
        otB = wpool.tile([128, NIB, 17], DT.float32, tag="otB")
        wb = wcol[:].unsqueeze(2).broadcast_to([128, NIB, 16])
        nc.vector.tensor_tensor(otA[:, :, 0:16], P_ps[:, :, 0:16], wb, AL.mult)
        # MLP: a = x ch0:16, b = attended
        nc.vector.tensor_tensor(otA[:, :, 16:32], otA[:, :, 0:16], xq, AL.add)
        nc.vector.tensor_tensor(otA[:, :, 32:48], otA[:, :, 0:16], xq, AL.mult)
        # count: c1*W_o1 = (T - g) * (W_o1/4), C psum already holds T - g
        nc.vector.scalar_tensor_tensor(otB[:, :, 0], P_ps[:, :, 17], cwo1,
                                       wcol[:], AL.mult, AL.bypass)
        nc.vector.tensor_tensor(otB[:, :, 1:17], otA[:, :, 0:16], xq, AL.is_lt)

        # stores ride the Act queue set: a tiny Act fence absorbs the
        # xbar mode-switch wait (first regular DMA after the transposes), so
        # each store carries only its DVE data-dep sem (walrus allows one)
        fence = cpool.tile([1, 128], DT.uint16)
        fdma = nc.scalar.dma_start(fence[0:1, :], blob_d[0:1, :])
        add_dep_helper(fdma.ins, last_tdma.ins, reason="absorb xbar sem")
        sdma = nc.scalar.dma_start(outa_d[:], otA[:])
        add_dep_helper(sdma.ins, fdma.ins, sync=False, reason="queue order")
        # absorb storeB's DVE data dep into an Act engine op so the store's
        # single wait slot is free for its queue-predecessor sem
        babs = cpool.tile([1, 1], DT.float32)
        nc.scalar.copy(babs[:], otB[0:1, 0, 16:17])
        sdmb = nc.scalar.dma_start(outb_d[:], otB[:])
        add_dep_helper(sdmb.ins, sdma.ins, sync=False, reason="queue order")

    return nc


_cached = {}


def _prepare_in_maps(x, mask0, mask1, W_o0, W_o1):
    x = np.asarray(x, dtype=np.float32)
    m0 = np.asarray(mask0).view(np.uint8)
    m1 = np.asarray(mask1).view(np.uint8)
    W_o0 = np.asarray(W_o0, dtype=np.float32)
    W_o1 = np.asarray(W_o1, dtype=np.float32)

    # nibble pack: byte jj = m0[2jj] + 2*m0[2jj+1] + 4*(m1[2jj]+m1[2jj+1])
    packed = (m0[..., 0::2] + (m0[..., 1::2] << 1)
              + ((m1[..., 0::2] + m1[..., 1::2]) << 2))        # (B, S, S//2) u8

    # u = values through the head-0 output projection; hi/lo split, x512
    u = x[:, :, 0:16] @ W_o0.T                                 # (B, S, 16) f32
    u_hi = u.astype(ml_dtypes.bfloat16).astype(np.float32)
    u_lo = u - u_hi

    # key index per (partition, chunk, byte-lane, parity)
    p_i = np.arange(128)[:, None, None, None]
    t_i = np.arange(NCH)[None, :, None, None]
    par_i = np.arange(2)[None, None, :, None]
    e_i = np.arange(2)[None, None, None, :]
    J = 512 * t_i + 4 * p_i + 2 * par_i + e_i                  # [128,4,2,2]

    # cone: +512 (raw bytes -> g + 4*c1); the -g side is folded into w3 col 17
    cone = np.full((128, 1), 512.0, dtype=ml_dtypes.bfloat16)
    cwo1 = np.full((128, 1), float(W_o1[0, 0]) / 4.0, dtype=np.float32)

    blobs = []
    for b in range(B):
        w3 = np.zeros((128, NCH, 2, 2, 2, 18), dtype=ml_dtypes.bfloat16)
        w3[..., 0, 0:16] = (512.0 * u_hi[b][J]).astype(ml_dtypes.bfloat16)
        w3[..., 1, 0:16] = (512.0 * u_lo[b][J]).astype(ml_dtypes.bfloat16)
        w3[..., 0, 16] = 512.0                                 # ones -> c0
        w3[..., 0, 17] = -512.0 * (1.0 + e_i[0, 0])            # -g accumulation
        blobs.append(w3)

    in_maps = []
    for c in range(8):
        b, h = divmod(c, 2)
        sl = slice(QH * h, QH * (h + 1))
        xq = np.ascontiguousarray(
            x[b, sl, 0:16].reshape(NIB, 128, 16).transpose(1, 0, 2))
        blob = np.zeros((128, 1696), np.uint8)
        blob[:, 0:1152] = blobs[b].reshape(128, 576).view(np.uint8)
        blob[:, 1152:1664] = xq.reshape(128, 128).view(np.uint8)
        blob[:, 1664:1668] = cwo1.view(np.uint8)
        blob[:, 1668:1670] = cone.view(np.uint8)
        blob_t = np.ascontiguousarray(blob.view(np.uint16).T)  # [848, 128]
        in_maps.append({
            "mp": np.ascontiguousarray(packed[b, sl, :]).view(np.uint16),
            "blob": blob_t,
        })
    return in_maps


def kernel(x, mask0, mask1, W_o0, W_o1):
    if "nc" not in _cached:
        _cached["nc"] = _build_program()
    nc = _cached["nc"]
    in_maps = _prepare_in_maps(x, mask0, mask1, W_o0, W_o1)
    res = run_bass_kernel_spmd(nc, in_maps, list(range(8)))
    _cached["last_results"] = res
    out = np.array(np.asarray(x, dtype=np.float32), copy=True)
    for c in range(8):
        b, h = divmod(c, 2)
        sl = slice(QH * h, QH * (h + 1))
        ra = res.results[c]["outa"].transpose(1, 0, 2).reshape(QH, 48)
        rb = res.results[c]["outb"].transpose(1, 0, 2).reshape(QH, 17)
        out[b, sl, 16:32] = ra[:, 0:16]
        out[b, sl, 48:64] = ra[:, 16:32]
        out[b, sl, 64:80] = ra[:, 32:48]
        out[b, sl, 32] = rb[:, 0]
        out[b, sl, 80:96] = rb[:, 1:17]
    return out


# revision 40
# speedup vs baseline: 1.2132x; 1.2132x over previous
"""Trainium2 Bass kernel for the sparse-attention CompiledTransformerLayer.

Math (derived from the reference):
  c0 = rowsum(mask0); attended = (mask0 @ x[:,:,0:16]) * r/(1-r), r = 1/(1+c0)
  out ch16:32 = attended @ W_o0.T
  out ch32    = c1 * W_o1[0,0], c1 = rowsum(mask1)
  out ch48:64 = a + b; 64:80 = a*b; 80:96 = (a > b), a = x ch0:16, b = ch16:32
  all other channels pass through from x (merged on the host).

Sharding: 8 cores = 4 batches x 2 query-halves (1024 queries each).

Key tricks:
  - nibble packing: the host packs BOTH masks for two adjacent keys into one
    byte  p = m0[2j] + 2*m0[2j+1] + 4*(m1[2j]+m1[2j+1]).  fp8e4 decodes bytes
    0..15 exactly as k*2^-9, so one transposed load (1MB/core instead of 4MB)
    carries all mask information.
  - the packed bytes are DMA-transposed as uint16 (HWDGE xbar), then two DVE
    shift/and ops recover the mask0 even/odd key bit-planes (values 0x00/0x01
    = fp8 0 / 2^-9).
  - matmuls are flipped vs the usual attention layout: a 128x128 mask^T block
    is the *stationary* operand and the 34-wide value vector
    [u_hi | u_lo | ones | gones] is the *moving* operand, so each matmul only
    streams 34 columns.  PSUM ends up as [query, channel] - no transposes.
  - u = x[:,:,0:16] @ W_o0.T is precomputed on the host, split hi+lo in bf16
    (scaled by 512 to cancel the fp8 2^-9), giving ~f32 matmul precision.
  - c1 falls out linearly: a raw-packed-byte matmul against ones gives
    T = g + 4*c1 where g = c0_even + 2*c0_odd comes from the 34th moving
    column (weight 1 on the even plane, 2 on the odd plane).
"""
import sys
sys.path.insert(0, "/opt/trn_rl_repo")
import numpy as np
import ml_dtypes

import concourse.bass as bass
import concourse.mybir as mybir
from concourse import tile
from concourse.bass_utils import run_bass_kernel_spmd
from concourse.vector_clock import ScopedClock, VectorClock
from concourse.tile import add_dep_helper

B, S, D = 4, 2048, 128
QH = S // 2              # queries per core
NCH = 4                  # transpose chunks (128 u16 cols = 512 keys each)
NIB = QH // 128          # query blocks per core (8)
DT = mybir.dt
AL = mybir.AluOpType

# walrus codegen rejects instructions with many sem waits; the Tile tail
# drain accumulates one wait per touched proc. Emit one single-wait drain
# per proc instead.
def _patched_dab(self, tick_clock, wait_clock):
    ticks = list(tick_clock.global_clock)
    for i, t in enumerate(ticks):
        if t <= 0:
            continue
        part = [t if j == i else 0 for j, t in enumerate(ticks)]
        d = self.nc.sync.drain()
        wait_clock.add_sem_waits(d.ins, ScopedClock({None: VectorClock(part)}))
    self.nc.sync.drain()
    self.nc.all_engine_barrier()
    popped = self.nc._tile_sem_poison_stack.pop()
    assert popped is self._sem_poison
    self.nc.clear_and_free_semaphores(list(self.sems.allocated().values()))
    self.nc.all_engine_barrier()
tile.TileContext._drain_and_barrier = _patched_dab


def _build_program():
    nc = bass.Bass()
    mp_d = nc.declare_dram_parameter("mp", [QH, S // 4], DT.uint16, isOutput=False)
    c0b_d = nc.declare_dram_parameter("c0b", [QH + 560, 128], DT.uint16, isOutput=False)
    outa_d = nc.declare_dram_parameter("outa", [128, NIB, 48], DT.float16, isOutput=True)
    outb_d = nc.declare_dram_parameter("outb", [128, NIB, 17], DT.float16, isOutput=True)

    # (chunk, row0, rows, ib0): the tail key-chunk splits into progressively
    # smaller query groups so the post-last-transpose straggler is tiny
    PIECES = [(1, 0, QH, 0), (2, 0, QH, 0), (3, 0, QH // 2, 0),
              (3, QH // 2, QH // 4, 4), (3, 3 * QH // 4, 128, 6),
              (3, 896, 128, 7)]

    with tile.TileContext(nc) as tc, \
         tc.tile_pool(name="const", bufs=1) as cpool, \
         tc.tile_pool(name="masks", bufs=1) as mpool, \
         tc.tile_pool(name="work", bufs=1) as wpool, \
         tc.tile_pool(name="ps", bufs=1, space="PSUM") as ps:

        # every input load goes through the xbar (transpose DMA): the first
        # transpose carries mask chunk 0 plus the small-input blob (w3, xq,
        # cwo1, cone) pre-transposed by the host.
        c0b = mpool.tile([128, QH + 560], DT.uint16, tag="c0b", name="c0b")
        nc.sync.dma_start(c0b[:], c0b_d[:], transpose=True)
        # DVE-launder the blob so every consumer dep collapses onto DVE sems
        blob2 = cpool.tile([128, 548], DT.uint16)
        nc.vector.tensor_copy(blob2[:], c0b[:, QH:QH + 548])
        w3 = blob2[:, 0:288].bitcast(DT.float16).rearrange(
            "p (t a b d) -> p t a b d", t=NCH, a=2, b=2, d=18)
        xq = blob2[:, 288:544].bitcast(DT.float32).rearrange(
            "p (i c) -> p i c", i=NIB, c=16)
        cwo1 = blob2[:, 544:546].bitcast(DT.float32)
        cone = blob2[:, 546:547].bitcast(DT.float16)

        # start=True resets psum at larger-than-region granularity, so zero the
        # whole accumulator with one dummy all-zeros matmul and use start=False
        # (pure accumulate) for every real matmul.  One tile holds S (cols
        # 0:16 att, 16 c0) and C (col 17, T - g) so all writers are PE.
        P_ps = ps.tile([128, NIB, 18], DT.float32, tag="P", name="P")
        zmv = cpool.tile([128, 160], DT.bfloat16)
        nc.vector.memset(zmv[:], 0.0)
        nc.tensor.matmul(P_ps[:].rearrange("p a b -> p (a b)"), zmv[:, 0:128],
                         zmv[:, 0:NIB * 18], start=True, stop=False,
                         skip_group_check=True)

        def emit_piece(t, mt_ap, ib0, nib, pi, last):
            rows = mt_ap.shape[1]
            # bit-plane extracts: even keys = bit0, odd keys = bit1 (per byte)
            ev = mpool.tile([128, rows], DT.uint16, tag=f"ev{pi}", name=f"ev{pi}")
            nc.vector.tensor_scalar(ev[:], mt_ap, 0x0101, 0, AL.bitwise_and,
                                    AL.bitwise_or)
            od = mpool.tile([128, rows], DT.uint16, tag=f"od{pi}", name=f"od{pi}")
            odx = nc.vector.tensor_scalar(od[:], mt_ap, 1, 0x0101,
                                          AL.logical_shift_right, AL.bitwise_and)

            mr = mt_ap.bitcast(DT.float8e4).rearrange("p (i two) -> p i two", two=2)
            evr = ev[:].bitcast(DT.float8e4).rearrange("p (i two) -> p i two", two=2)
            odr = od[:].bitcast(DT.float8e4).rearrange("p (i two) -> p i two", two=2)

            for par in range(2):
                # 18-wide fp16 moving: [u (16) | ones -> c0 | -(1+e) ones -> -g]
                for e, pl in ((0, evr), (1, odr)):
                    stops = (last and par == 1 and e == 1)
                    mv = w3[:, t, par, e, :]
                    for k in range(nib):
                        nc.tensor.matmul(
                            P_ps[:, ib0 + k, 0:18],
                            pl[:, 128 * k:128 * (k + 1), par], mv,
                            start=False,
                            stop=(stops and k == nib - 1),
                            skip_group_check=True)
                # raw packed bytes vs ones: C += g + 4*c1
                for k in range(nib):
                    cmm = nc.tensor.matmul(
                        P_ps[:, ib0 + k, 17:18], mr[:, 128 * k:128 * (k + 1), par],
                        cone[:, 0:1],
                        start=False,
                        stop=(last and par == 1 and k == nib - 1),
                        skip_group_check=True)
                    # route deps through the chunk's DVE extract so the wait
                    # set collapses to a single DVE sem (covers mt + cone)
                    add_dep_helper(cmm.ins, odx.ins, reason="chunk ready")

        emit_piece(0, c0b[:, 0:QH], 0, NIB, 0, False)
        last_tdma = None
        for pi, (t, row0, rows, ib0) in enumerate(PIECES):
            mt = mpool.tile([128, rows], DT.uint16, tag=f"mt{pi}", name=f"mt{pi}")
            last_tdma = nc.sync.dma_start(
                mt[:], mp_d[row0:row0 + rows, 128 * t:128 * (t + 1)],
                transpose=True)
            emit_piece(t, mt[:], ib0, rows // 128, pi + 1,
                       pi == len(PIECES) - 1)

        # ---- tail: scale + MLP, all in [128 queries, NIB, ch] layout ------
        # attended = S * w with w = 1/max(c0, 1): exact for c0 >= 1, and for
        # c0 == 0 S is exactly 0 so any finite w gives the reference 0.
        mcol = wpool.tile([128, NIB], DT.float32, tag="mcol")
        nc.vector.tensor_scalar_max(mcol[:], P_ps[:, :, 16], 1.0)
        wcol = wpool.tile([128, NIB], DT.float32, tag="wcol")
        nc.vector.reciprocal(wcol[:], mcol[:])

        # otA: [atts 16 | a+b 16 | a*b 16]; otB: [count 1 | a>b 16] (fp16:
        # count <= 2047 and the 0/1 comparison bits are exact in fp16)
        otA = wpool.tile([128, NIB, 48], DT.float16, tag="otA")
        otB = wpool.tile([128, NIB, 17], DT.float16, tag="otB")
        atts = wpool.tile([128, NIB, 16], DT.float32, tag="atts")
        wb = wcol[:].unsqueeze(2).broadcast_to([128, NIB, 16])
        nc.vector.tensor_tensor(atts[:], P_ps[:, :, 0:16], wb, AL.mult)
        nc.vector.tensor_copy(otA[:, :, 0:16], atts[:])
        # MLP: a = x ch0:16, b = attended
        nc.vector.tensor_tensor(otA[:, :, 16:32], atts[:], xq, AL.add)
        nc.vector.tensor_tensor(otA[:, :, 32:48], atts[:], xq, AL.mult)
        # count: c1*W_o1 = (T - g) * (W_o1/4), C psum already holds T - g
        nc.vector.scalar_tensor_tensor(otB[:, :, 0], P_ps[:, :, 17], cwo1,
                                       wcol[:], AL.mult, AL.bypass)
        nc.vector.tensor_tensor(otB[:, :, 1:17], atts[:], xq, AL.is_lt)

        # stores ride the Act queue set.  A tiny Act fence absorbs the xbar
        # mode-switch wait; Act engine copies absorb each store's DVE data
        # dep, leaving the single allowed sem wait for the queue-predecessor.
        fence = cpool.tile([1, 128], DT.uint16)
        fdma = nc.scalar.dma_start(fence[0:1, :], c0b_d[0:1, :])
        add_dep_helper(fdma.ins, last_tdma.ins, reason="absorb xbar sem")
        babs = cpool.tile([1, 2], DT.float16)
        nc.scalar.copy(babs[0:1, 0:1], otA[0:1, 0, 47:48])
        sdma = nc.scalar.dma_start(outa_d[:], otA[:])
        add_dep_helper(sdma.ins, fdma.ins, sync=False, reason="queue order")
        nc.scalar.copy(babs[0:1, 1:2], otB[0:1, 0, 16:17])
        sdmb = nc.scalar.dma_start(outb_d[:], otB[:])
        add_dep_helper(sdmb.ins, sdma.ins, sync=False, reason="queue order")

    return nc


_cached = {}


def _prepare_in_maps(x, mask0, mask1, W_o0, W_o1):
    x = np.asarray(x, dtype=np.float32)
    m0 = np.asarray(mask0).view(np.uint8)
    m1 = np.asarray(mask1).view(np.uint8)
    W_o0 = np.asarray(W_o0, dtype=np.float32)
    W_o1 = np.asarray(W_o1, dtype=np.float32)

    # nibble pack: byte jj = m0[2jj] + 2*m0[2jj+1] + 4*(m1[2jj]+m1[2jj+1])
    packed = (m0[..., 0::2] + (m0[..., 1::2] << 1)
              + ((m1[..., 0::2] + m1[..., 1::2]) << 2))        # (B, S, S//2) u8

    # u = values through the head-0 output projection, fp16 x512
    u = x[:, :, 0:16] @ W_o0.T                                 # (B, S, 16) f32
    u16 = (512.0 * u).astype(np.float16)

    # key index per (partition, chunk, byte-lane, parity)
    p_i = np.arange(128)[:, None, None, None]
    t_i = np.arange(NCH)[None, :, None, None]
    par_i = np.arange(2)[None, None, :, None]
    e_i = np.arange(2)[None, None, None, :]
    J = 512 * t_i + 4 * p_i + 2 * par_i + e_i                  # [128,4,2,2]

    cwo1 = np.full((128, 1), float(W_o1[0, 0]) / 4.0, dtype=np.float32)
    cone = np.full((128, 1), 512.0, dtype=np.float16)

    w3s = []
    for b in range(B):
        w3 = np.zeros((128, NCH, 2, 2, 18), dtype=np.float16)
        w3[..., 0:16] = u16[b][J]
        w3[..., 16] = 512.0                                    # ones -> c0
        w3[..., 17] = -512.0 * (1.0 + e_i[0, 0])               # -g accumulation
        w3s.append(w3)

    in_maps = []
    for c in range(8):
        b, h = divmod(c, 2)
        sl = slice(QH * h, QH * (h + 1))
        xq = np.ascontiguousarray(
            x[b, sl, 0:16].reshape(NIB, 128, 16).transpose(1, 0, 2))
        blob = np.zeros((128, 1120), np.uint8)
        blob[:, 0:576] = w3s[b].reshape(128, 288).view(np.uint8)
        blob[:, 576:1088] = xq.reshape(128, 128).view(np.uint8)
        blob[:, 1088:1092] = cwo1.view(np.uint8)
        blob[:, 1092:1094] = cone.view(np.uint8)
        blob_t = blob.view(np.uint16).T                        # [560, 128]
        mp = np.ascontiguousarray(packed[b, sl, :]).view(np.uint16)
        c0b = np.ascontiguousarray(
            np.concatenate([mp[:, 0:128], blob_t], axis=0))    # [1584, 128]
        in_maps.append({"mp": mp, "c0b": c0b})
    return in_maps


def kernel(x, mask0, mask1, W_o0, W_o1):
    if "nc" not in _cached:
        _cached["nc"] = _build_program()
    nc = _cached["nc"]
    in_maps = _prepare_in_maps(x, mask0, mask1, W_o0, W_o1)
    res = run_bass_kernel_spmd(nc, in_maps, list(range(8)))
    _cached["last_results"] = res
    out = np.array(np.asarray(x, dtype=np.float32), copy=True)
    for c in range(8):
        b, h = divmod(c, 2)
        sl = slice(QH * h, QH * (h + 1))
        ra = res.results[c]["outa"].astype(np.float32).transpose(1, 0, 2).reshape(QH, 48)
        rb = res.results[c]["outb"].astype(np.float32).transpose(1, 0, 2).reshape(QH, 17)
        out[b, sl, 16:32] = ra[:, 0:16]
        out[b, sl, 48:64] = ra[:, 16:32]
        out[b, sl, 64:80] = ra[:, 32:48]
        out[b, sl, 32] = rb[:, 0]
        out[b, sl, 80:96] = rb[:, 1:17]
    return out
